# revision 1
# baseline (speedup 1.0000x reference)
"""Trainium2 Bass kernel: 3x3 conv (N=16, C_in=16, C_out=64, H=W=256, pad=1).

Strategy (8 NeuronCores, data-parallel over batch N -> 2 images/core):
  - Host pads x to [2,16,258,258] (zero ring) so the kernel has no edge cases.
  - Per 64-row "superstep": two 32-row strips (A,B) are stacked on SBUF
    partitions 0-47 / 48-95 as (kh, ci) im2col slabs; kh-shifted blocks are
    built with two SBUF->SBUF DMA copies from the center block.
  - One matmul per kw tap (3 total, PSUM-accumulated) with a [96,128]
    block-diagonal fp32r weight matrix computes both strips' 64 output
    channels for 512 pixels (2 rows x 256) in one instruction; kw shifts are
    pure free-dim offsets into the 258-pitch slab (gap columns are zero).
  - PSUM -> SBUF evacuation on VectorE, 512KB store DMAs.
"""

import sys

if "/opt/trn_rl_repo" not in sys.path:
    sys.path.insert(0, "/opt/trn_rl_repo")

import numpy as np

import concourse.bacc as bacc
import concourse.bass as bass
import concourse.mybir as mybir
import concourse.tile as tile
from concourse.bass_utils import run_bass_kernel_spmd

N_FULL, CI, CO, H, W_SP = 16, 16, 64, 256, 256
NCORES = 8
NB = N_FULL // NCORES          # batches per core
HP, WP = H + 2, W_SP + 2       # padded image dims
SLOT = WP                      # 258: one row-slot in the slab (z x0..x255 z)
RSTRIP = 32                    # output rows per strip
SLOTS = RSTRIP + 2             # row-slots per strip slab (rows + 2 halo)
NSS = H // (2 * RSTRIP)        # supersteps per image (4)
NBANK = RSTRIP // 2            # PSUM banks per superstep (16, pool rotates 8)
F32 = mybir.dt.float32
F32R = mybir.dt.float32r

_CACHE = {}


def _build(reps: int = 1):
    nc = bacc.Bacc("TRN2", target_bir_lowering=False, debug=False)
    x_d = nc.dram_tensor("xp", [NB, CI, HP, WP], F32, kind="ExternalInput").ap()
    w_d = nc.dram_tensor("wts", [3, 96, 128], F32, kind="ExternalInput").ap()
    o_d = nc.dram_tensor("out", [NB, CO, H, W_SP], F32, kind="ExternalOutput").ap()

    # out[n, co, (t, s, j, r), w] view for per-(superstep, strip, evac) stores
    o_v = o_d.rearrange("n c (t s j r) w -> n t s j c (r w)", t=NSS, s=2, j=4)

    xe_n = CI * HP * WP        # x_pad element strides
    xe_c = HP * WP
    xe_h = WP

    with tile.TileContext(nc) as tc:
        with (
            tc.tile_pool(name="wp", bufs=1) as wpool,
            tc.tile_pool(name="slab", bufs=4) as slabpool,
            tc.tile_pool(name="evac", bufs=6) as evacpool,
            tc.tile_pool(name="ps", bufs=8, space="PSUM") as pspool,
        ):
            # weights tile (loads emitted after the prologue slab loads so
            # the first center loads aren't stuck behind Q7 emission)
            wsb = wpool.tile([96, 3 * 128], F32R)

            def build_slab(n, t, dma_copies=False):
                # slab partition layout: [block0(A,B) | center(A,B) | block2(A,B)]
                # i.e. partition = kh*32 + strip*16 + ci.  Center loads are
                # per-strip (SBUF-side DMA APs must stay 2D single-level);
                # the kh=0 / kh=2 blocks are each ONE row-shifted 32-partition
                # SBUF->SBUF copy of both centers.
                h0 = 2 * RSTRIP * t
                slab = slabpool.tile([96, SLOTS * SLOT], F32R, tag="slab")
                sf = slab[:]
                for strip in range(2):
                    src = bass.AP(
                        x_d.tensor,
                        n * xe_n + (h0 + strip * RSTRIP) * xe_h,
                        [[xe_c, CI], [1, SLOTS * SLOT]],
                    )
                    nc.gpsimd.dma_start(sf[32 + 16 * strip : 48 + 16 * strip, :], src)
                copy_op = nc.sync.dma_start if dma_copies else nc.scalar.copy
                copy_op(
                    sf[0:32, SLOT : (SLOTS - 1) * SLOT],
                    sf[32:64, 0 : (SLOTS - 2) * SLOT],
                )
                copy_op(
                    sf[64:96, SLOT : (SLOTS - 1) * SLOT],
                    sf[32:64, 2 * SLOT : SLOTS * SLOT],
                )
                return slab

            def compute(n, t, slab):
                su = slab[:].rearrange("p (u e) -> p u e", u=SLOTS)
                for j in range(4):
                    evac = evacpool.tile([128, 4 * 512], F32, tag="evac")
                    for bb in range(4):
                        b = 4 * j + bb
                        ps = pspool.tile([128, 512], F32, tag="ps")
                        for kw in range(3):
                            rhs = su[:, 2 * b + 1 : 2 * b + 3, kw : kw + 256]
                            nc.tensor.matmul(
                                ps[:],
                                wsb[:, kw * 128 : (kw + 1) * 128],
                                rhs,
                                start=(kw == 0),
                                stop=(kw == 2),
                            )
                        nc.vector.tensor_copy(
                            evac[:, bb * 512 : (bb + 1) * 512], ps[:]
                        )
                    for strip in range(2):
                        nc.sync.dma_start(
                            o_v[n, t, strip, j],
                            evac[strip * 64 : (strip + 1) * 64, :],
                        )

            # software pipeline with two-superstep lookahead on slab builds
            LOOK = 3
            steps = [(n, t) for _ in range(reps) for n in range(NB) for t in range(NSS)]
            slabs = {}
            for k in range(min(LOOK, len(steps))):
                slabs[steps[k]] = build_slab(*steps[k], dma_copies=(k < 2))
                if k == 0:
                    for kw in range(3):
                        nc.gpsimd.dma_start(
                            wsb[:, kw * 128 : (kw + 1) * 128], w_d[kw]
                        )
            for i, (n, t) in enumerate(steps):
                if i + LOOK < len(steps):
                    slabs[steps[i + LOOK]] = build_slab(*steps[i + LOOK])
                compute(n, t, slabs.pop((n, t)))

    nc.compile()
    return nc


def _prep_weights(W: np.ndarray) -> np.ndarray:
    # lhsT[kw][kh*32 + strip*16 + ci, strip*64 + co] = W[co, ci, kh, kw]
    wts = np.zeros((3, 96, 128), dtype=np.float32)
    blk = np.ascontiguousarray(W.transpose(3, 2, 1, 0))  # [kw, kh, ci, co]
    for kh in range(3):
        for strip in range(2):
            wts[:, kh * 32 + strip * 16 : kh * 32 + (strip + 1) * 16,
                strip * 64 : (strip + 1) * 64] = blk[:, kh]
    return wts


def kernel(x: np.ndarray, W: np.ndarray) -> np.ndarray:
    assert x.shape == (N_FULL, CI, H, W_SP) and W.shape == (CO, CI, 3, 3)
    # BASS_TRACE without the axon NTFF hook module would crash the run path;
    # disable tracing only when the hook is genuinely unavailable.
    try:
        import antenv.axon_hooks  # noqa: F401
    except Exception:
        import os

        os.environ.setdefault("BASS_NEVER_TRACE", "1")
    if "nc" not in _CACHE:
        _CACHE["nc"] = _build()
    nc = _CACHE["nc"]

    wts = _prep_weights(np.asarray(W, dtype=np.float32))
    xs = np.asarray(x, dtype=np.float32).reshape(NCORES, NB, CI, H, W_SP)
    in_maps = []
    for i in range(NCORES):
        xp = np.zeros((NB, CI, HP, WP), dtype=np.float32)
        xp[:, :, 1 : H + 1, 1 : W_SP + 1] = xs[i]
        in_maps.append({"xp": xp, "wts": wts})

    res = run_bass_kernel_spmd(nc, in_maps, list(range(NCORES)))
    out = np.concatenate([res.results[i]["out"] for i in range(NCORES)], axis=0)
    return out



# revision 32
# speedup vs baseline: 1.7701x; 1.7701x over previous
"""Trainium2 Bass kernel: 3x3 conv (N=16, C_in=16, C_out=64, H=W=256, pad=1).

Strategy (8 NeuronCores, data-parallel over batch N -> 2 images/core):
  - All device I/O in fp16 (host converts): DMA floor ~21MB/core.
  - Per 64-row strip: slab [128 partitions = (kw d in {0,1}, row-slot s in
    {0..3}, ci)] holds 4-row groups at stride 2; partition (d,s,ci) at free
    (g, j) = xpad[ci, h0+2g+s-1, j+d].  One HBM load fills (d=0, s in {0,1});
    three DVE copies (4x perf mode) build the rest from it.
  - 2 matmul passes per psum tile (vs 3 in the kh-block scheme): pass1
    contracts all 128 partitions (kw0+kw1 taps), pass2 contracts the kw1
    block read at +1px (= kw2 taps).  M = 128 = (row-phase ph in {0,1}) x
    64 channels -> 131072 psum rows total = PE structural floor.
  - PSUM -> SBUF evac with fp32->fp16 convert split across Scalar/GpSimd;
    batched 64-partition stores (even/odd output rows).
"""

import sys

if "/opt/trn_rl_repo" not in sys.path:
    sys.path.insert(0, "/opt/trn_rl_repo")

import numpy as np

import concourse.bacc as bacc
import concourse.bass as bass
import concourse.mybir as mybir
import concourse.tile as tile
from concourse.bass_utils import run_bass_kernel_spmd

N_FULL, CI, CO, H, W_SP = 16, 16, 64, 256, 256
NCORES = 8
NB = N_FULL // NCORES          # images per core
HP, WP = H + 2, W_SP + 2       # padded image dims (258)
RSTRIP = 64                    # output rows per strip
NSS = H // RSTRIP              # strips per image (4)
G = RSTRIP // 2                # row-pairs per strip (32)
GH = G + 1                     # loaded groups (incl. halo group)
NT = G // 2                    # psum tiles per strip (16)
F32 = mybir.dt.float32
F16 = mybir.dt.float16

_CACHE = {}


def _build():
    nc = bacc.Bacc("TRN2", target_bir_lowering=False, debug=False)
    EB = 8                     # psum tiles per evac batch (32 output rows)
    NEB = NT // EB             # evac batches per strip (2)

    x_d = nc.dram_tensor("xp", [NB, CI, HP, WP], F16, kind="ExternalInput").ap()
    w_d = nc.dram_tensor("wts", [192, 128], F16, kind="ExternalInput").ap()
    # device-layout output: [n, strip, batch, (ph,co), (t', gi, j)];
    # host un-permutes to NCHW (out row = 64t + 32b + 4t' + 2gi + ph)
    o_d = nc.dram_tensor(
        "out", [NB, NSS, NEB, 128, EB * 512], F16, kind="ExternalOutput"
    ).ap()

    xe_n = CI * HP * WP        # x_pad element strides
    xe_c = HP * WP
    xe_h = WP

    with tile.TileContext(nc) as tc:
        with (
            tc.tile_pool(name="wp", bufs=1) as wpool,
            tc.tile_pool(name="slab", bufs=4) as slabpool,
            tc.tile_pool(name="evac", bufs=4) as evacpool,
            tc.tile_pool(name="ps", bufs=4, space="PSUM") as pspool,
        ):
            wsb = wpool.tile([128, 256], F16)
            # w1 = wsb[:, 0:128] (pass1, all 128 partitions)
            # w2 = wsb[64:128, 128:256] (pass2, kw1 block partitions)
            # (loads emitted in the prologue, after slab0's loads)

            def copies(sv, g0, g1):
                # c1:  kw0_s23[g] = kw0_s01[g+1]      (rows 2g+1, 2g+2)
                # c23: kw1[g,j]   = kw0[g,j+1]        (64-partition copy)
                nc.vector.tensor_copy(
                    sv[32:64, g0:g1, 0:WP],
                    sv[0:32, g0 + 1 : g1 + 1, 0:WP],
                )
                nc.vector.tensor_copy(
                    sv[64:128, g0:g1, 0 : WP - 1],
                    sv[0:64, g0:g1, 1:WP],
                )

            def load_slab(n, t):
                # slab covers the whole strip: groups 0..G-1 (+ halo slot G)
                slab = slabpool.tile([128, GH * WP], F16, tag="slab")
                sf = slab[:]
                for s in range(2):
                    src = bass.AP(
                        x_d.tensor,
                        n * xe_n + (RSTRIP * t + s) * xe_h,
                        [[xe_c, CI], [2 * xe_h, GH], [1, WP]],
                    )
                    nc.sync.dma_start(sf[16 * s : 16 * s + 16, :], src)
                return slab

            def compute(n, t, slab, all_scalar=False, fine_stores=False):
                sv = slab[:].rearrange("p (g j) -> p g j", j=WP)
                for eb in range(4):
                    evac = evacpool.tile([128, 2048], F16, tag="evac")
                    for pth in range(2):
                        pt = 2 * eb + pth
                        ps = pspool.tile([128, 1024], F32, tag="ps")
                        for q in range(2):
                            g0 = 2 * (2 * pt + q)
                            nc.tensor.matmul(
                                ps[:, q * 512 : (q + 1) * 512],
                                wsb[0:128, 0:128],
                                sv[0:128, g0 : g0 + 2, 0:256],
                                start=True,
                                stop=False,
                            )
                            nc.tensor.matmul(
                                ps[:, q * 512 : (q + 1) * 512],
                                wsb[64:128, 128:256],
                                sv[64:128, g0 : g0 + 2, 1:257],
                                start=False,
                                stop=True,
                            )
                        dve = (pt % 4 == 3) and not all_scalar
                        op = nc.vector.tensor_copy if dve else nc.scalar.copy
                        op(evac[:, pth * 1024 : (pth + 1) * 1024], ps[:])
                        if fine_stores:
                            dst = bass.AP(
                                o_d.tensor,
                                (((n * NSS + t) * 4 + eb) * 128 * 2048
                                 + pth * 1024),
                                [[2048, 128], [1, 1024]],
                            )
                            nc.sync.dma_start(
                                dst, evac[:, pth * 1024 : (pth + 1) * 1024]
                            )
                    if not fine_stores:
                        dst = bass.AP(
                            o_d.tensor,
                            ((n * NSS + t) * 4 + eb) * 128 * 2048,
                            [[2048, 128], [1, 2048]],
                        )
                        nc.sync.dma_start(dst, evac[:])

            # --- software pipeline ---------------------------------------
            # Loads run 2 steps ahead (SP queue, ahead of stores); copies run
            # 1 step ahead and are emitted BEFORE each step's evacs so the
            # DVE never idles on matmul waits while copies are ready.
            steps = [(n, t) for n in range(NB) for t in range(NSS)]

            # prologue: slab 0 as two halves for a fast start
            slab0 = slabpool.tile([128, GH * WP], F16, tag="slab")
            sv0 = slab0[:].rearrange("p (g j) -> p g j", j=WP)
            HG = G // 2
            for half in range(2):
                g0 = half * HG
                for s in range(2):
                    src = bass.AP(
                        x_d.tensor,
                        (2 * g0 + s) * xe_h,
                        [[xe_c, CI], [2 * xe_h, HG + 1], [1, WP]],
                    )
                    nc.sync.dma_start(
                        slab0[:][16 * s : 16 * s + 16,
                                 g0 * WP : (g0 + HG + 1) * WP],
                        src,
                    )
                if half == 0:
                    nc.sync.dma_start(wsb[0:128, 0:128], w_d[0:128, :])
                    nc.sync.dma_start(wsb[64:128, 128:256], w_d[128:192, :])
                copies(sv0, g0, g0 + HG)
            slabs = {steps[0]: slab0}
            slabs[steps[1]] = load_slab(*steps[1])
            copies(slabs[steps[1]][:].rearrange("p (g j) -> p g j", j=WP),
                   0, G)
            for i, (n, t) in enumerate(steps):
                if i + 2 < len(steps):
                    slabs[steps[i + 2]] = load_slab(*steps[i + 2])
                if i + 1 < len(steps) and i >= 1:
                    copies(
                        slabs[steps[i + 1]][:].rearrange(
                            "p (g j) -> p g j", j=WP
                        ),
                        0, G,
                    )
                last = i == len(steps) - 1
                compute(n, t, slabs.pop((n, t)),
                        all_scalar=(i == 0), fine_stores=last)

    nc.compile()
    return nc


def _prep_weights(W: np.ndarray) -> np.ndarray:
    # lhsT layouts, stored stacked as [192, 128] then transposed on load:
    #   w1[(d,s,ci), (ph,co)] = W[co, ci, s-ph, d]    (rows 0..127)
    #   w2[(s,ci),   (ph,co)] = W[co, ci, s-ph, 2]    (rows 128..191)
    w = np.zeros((192, 128), dtype=np.float32)
    for s in range(4):
        for ph in range(2):
            kh = s - ph
            if not (0 <= kh <= 2):
                continue
            blk = W[:, :, kh, :]  # [co, ci, kw]
            for d in range(2):
                w[d * 64 + s * 16 : d * 64 + (s + 1) * 16,
                  ph * 64 : (ph + 1) * 64] = blk[:, :, d].T
            w[128 + s * 16 : 128 + (s + 1) * 16,
              ph * 64 : (ph + 1) * 64] = blk[:, :, 2].T
    return w.astype(np.float16)


def _prep_inputs(x: np.ndarray, W: np.ndarray) -> list[dict]:
    wts = _prep_weights(np.asarray(W, dtype=np.float32))
    xs = np.asarray(x, dtype=np.float32).reshape(NCORES, NB, CI, H, W_SP)
    in_maps = []
    for i in range(NCORES):
        xp = np.zeros((NB, CI, HP, WP), dtype=np.float16)
        xp[:, :, 1 : H + 1, 1 : W_SP + 1] = xs[i]
        in_maps.append({"xp": xp, "wts": wts})
    return in_maps


def kernel(x: np.ndarray, W: np.ndarray) -> np.ndarray:
    assert x.shape == (N_FULL, CI, H, W_SP) and W.shape == (CO, CI, 3, 3)
    # BASS_TRACE without the axon NTFF hook module would crash the run path;
    # disable tracing only when the hook is genuinely unavailable.
    try:
        import antenv.axon_hooks  # noqa: F401
    except Exception:
        import os

        os.environ.setdefault("BASS_NEVER_TRACE", "1")
    if "nc" not in _CACHE:
        _CACHE["nc"] = _build()
    nc = _CACHE["nc"]

    in_maps = _prep_inputs(x, W)
    res = run_bass_kernel_spmd(nc, in_maps, list(range(NCORES)))
    parts = []
    for i in range(NCORES):
        dev = np.asarray(res.results[i]["out"], dtype=np.float32)
        # [n, t, eb, ph, co, h, gi, j] -> [n, co, (t eb h gi ph), j]
        # (out row = 64t + 16eb + 4h + 2gi + ph)
        dev = dev.reshape(NB, NSS, 4, 2, CO, 4, 2, 256)
        dev = dev.transpose(0, 4, 1, 2, 5, 6, 3, 7).reshape(NB, CO, H, W_SP)
        parts.append(dev)
    return np.concatenate(parts, axis=0)


# revision 58
# speedup vs baseline: 1.7741x; 1.0023x over previous
"""Trainium2 Bass kernel: 3x3 conv (N=16, C_in=16, C_out=64, H=W=256, pad=1).

Strategy (8 NeuronCores, data-parallel over batch N -> 2 images/core):
  - All device I/O in fp16 (host converts): DMA floor ~21MB/core.
  - Per 64-row strip: slab [128 partitions = (kw d in {0,1}, row-slot s in
    {0..3}, ci)] holds 4-row groups at stride 2; partition (d,s,ci) at free
    (g, j) = xpad[ci, h0+2g+s-1, j+d].  One HBM load fills (d=0, s in {0,1});
    three DVE copies (4x perf mode) build the rest from it.
  - 2 matmul passes per psum tile (vs 3 in the kh-block scheme): pass1
    contracts all 128 partitions (kw0+kw1 taps), pass2 contracts the kw1
    block read at +1px (= kw2 taps).  M = 128 = (row-phase ph in {0,1}) x
    64 channels -> 131072 psum rows total = PE structural floor.
  - PSUM -> SBUF evac with fp32->fp16 convert split across Scalar/GpSimd;
    batched 64-partition stores (even/odd output rows).
"""

import sys

if "/opt/trn_rl_repo" not in sys.path:
    sys.path.insert(0, "/opt/trn_rl_repo")

import numpy as np

import concourse.bacc as bacc
import concourse.bass as bass
import concourse.mybir as mybir
import concourse.tile as tile
from concourse.bass_utils import run_bass_kernel_spmd

N_FULL, CI, CO, H, W_SP = 16, 16, 64, 256, 256
NCORES = 8
NB = N_FULL // NCORES          # images per core
HP, WP = H + 2, W_SP + 2       # padded image dims (258)
RSTRIP = 64                    # output rows per strip
NSS = H // RSTRIP              # strips per image (4)
G = RSTRIP // 2                # row-pairs per strip (32)
GH = G + 1                     # loaded groups (incl. halo group)
NT = G // 2                    # psum tiles per strip (16)
F32 = mybir.dt.float32
F16 = mybir.dt.float16

_CACHE = {}


def _build():
    nc = bacc.Bacc("TRN2", target_bir_lowering=False, debug=False)
    EB = 8                     # psum tiles per evac batch (32 output rows)
    NEB = NT // EB             # evac batches per strip (2)

    x_d = nc.dram_tensor("xp", [NB, CI, HP, WP], F16, kind="ExternalInput").ap()
    w_d = nc.dram_tensor("wts", [192, 128], F16, kind="ExternalInput").ap()
    # device-layout output: [n, strip, batch, (ph,co), (t', gi, j)];
    # host un-permutes to NCHW (out row = 64t + 32b + 4t' + 2gi + ph)
    o_d = nc.dram_tensor(
        "out", [NB, NSS, NEB, 128, EB * 512], F16, kind="ExternalOutput"
    ).ap()

    xe_n = CI * HP * WP        # x_pad element strides
    xe_c = HP * WP
    xe_h = WP

    with tile.TileContext(nc) as tc:
        with (
            tc.tile_pool(name="wp", bufs=1) as wpool,
            tc.tile_pool(name="slab", bufs=4) as slabpool,
            tc.tile_pool(name="evac", bufs=4) as evacpool,
            tc.tile_pool(name="ps", bufs=4, space="PSUM") as pspool,
        ):
            wsb = wpool.tile([128, 256], F16)
            # w1 = wsb[:, 0:128] (pass1, all 128 partitions)
            # w2 = wsb[64:128, 128:256] (pass2, kw1 block partitions)
            # (loads emitted in the prologue, after slab0's loads)

            def copies(sv, g0, g1, pool_c1=0):
                # c1:  kw0_s23[g] = kw0_s01[g+1]      (rows 2g+1, 2g+2)
                #      (optionally first `pool_c1` groups on GpSimd)
                # c23: kw1[g,j]   = kw0[g,j+1]        (64-partition copy)
                if pool_c1:
                    nc.gpsimd.tensor_copy(
                        sv[32:64, g0 : g0 + pool_c1, 0:WP],
                        sv[0:32, g0 + 1 : g0 + pool_c1 + 1, 0:WP],
                    )
                nc.vector.tensor_copy(
                    sv[32:64, g0 + pool_c1 : g1, 0:WP],
                    sv[0:32, g0 + pool_c1 + 1 : g1 + 1, 0:WP],
                )
                nc.vector.tensor_copy(
                    sv[64:128, g0:g1, 0 : WP - 1],
                    sv[0:64, g0:g1, 1:WP],
                )

            def load_slab(n, t, eng=None):
                # slab covers the whole strip: groups 0..G-1 (+ halo slot G)
                slab = slabpool.tile([128, GH * WP], F16, tag="slab")
                sf = slab[:]
                for s in range(2):
                    src = bass.AP(
                        x_d.tensor,
                        n * xe_n + (RSTRIP * t + s) * xe_h,
                        [[xe_c, CI], [2 * xe_h, GH], [1, WP]],
                    )
                    (eng or nc.sync).dma_start(sf[16 * s : 16 * s + 16, :], src)
                return slab

            def load_slab_extras(slab, n, t, eng):
                # fill s23 / kw1 blocks straight from HBM (no DVE copies);
                # only worthwhile while the DMA queue still has slack
                sf = slab[:]
                base = n * xe_n + RSTRIP * t * xe_h
                for s in (2, 3):     # kw0_s23: x-rows 2g+s-1, g in 0..G-1
                    src = bass.AP(
                        x_d.tensor,
                        base + s * xe_h,
                        [[xe_c, CI], [2 * xe_h, G], [1, WP]],
                    )
                    eng.dma_start(
                        sf[16 * s : 16 * s + 16, 0 : G * WP], src
                    )
                sv = sf.rearrange("p (g j) -> p g j", j=WP)
                for s in range(4):   # kw1: same rows, +1 px, 257 cols
                    src = bass.AP(
                        x_d.tensor,
                        base + s * xe_h + 1,
                        [[xe_c, CI], [2 * xe_h, G], [1, WP - 1]],
                    )
                    eng.dma_start(
                        sv[64 + 16 * s : 80 + 16 * s, 0:G, 0 : WP - 1], src
                    )

            def compute(n, t, slab, all_scalar=False, fine_stores=False,
                        dve_evacs=(3, 7), ebs=range(4)):
                sv = slab[:].rearrange("p (g j) -> p g j", j=WP)
                for eb in ebs:
                    evac = evacpool.tile([128, 2048], F16, tag="evac")
                    for pth in range(2):
                        pt = 2 * eb + pth
                        ps = pspool.tile([128, 1024], F32, tag="ps")
                        for q in range(2):
                            g0 = 2 * (2 * pt + q)
                            nc.tensor.matmul(
                                ps[:, q * 512 : (q + 1) * 512],
                                wsb[0:128, 0:128],
                                sv[0:128, g0 : g0 + 2, 0:256],
                                start=True,
                                stop=False,
                            )
                            nc.tensor.matmul(
                                ps[:, q * 512 : (q + 1) * 512],
                                wsb[64:128, 128:256],
                                sv[64:128, g0 : g0 + 2, 1:257],
                                start=False,
                                stop=True,
                            )
                        dve = (pt in dve_evacs) and not all_scalar
                        op = nc.vector.tensor_copy if dve else nc.scalar.copy
                        op(evac[:, pth * 1024 : (pth + 1) * 1024], ps[:])
                        if fine_stores:
                            dst = bass.AP(
                                o_d.tensor,
                                (((n * NSS + t) * 4 + eb) * 128 * 2048
                                 + pth * 1024),
                                [[2048, 128], [1, 1024]],
                            )
                            nc.sync.dma_start(
                                dst, evac[:, pth * 1024 : (pth + 1) * 1024]
                            )
                    if not fine_stores:
                        dst = bass.AP(
                            o_d.tensor,
                            ((n * NSS + t) * 4 + eb) * 128 * 2048,
                            [[2048, 128], [1, 2048]],
                        )
                        nc.sync.dma_start(dst, evac[:])

            # --- software pipeline ---------------------------------------
            # Loads run 2 steps ahead (SP queue, ahead of stores); copies run
            # 1 step ahead and are emitted BEFORE each step's evacs so the
            # DVE never idles on matmul waits while copies are ready.
            steps = [(n, t) for n in range(NB) for t in range(NSS)]

            # prologue: slab 0 in quarters for a fast start (dep tracking is
            # region-precise, so early matmuls run on partial slabs);
            # high_priority pins these ahead of later copies in the scheduler
            slab0 = slabpool.tile([128, GH * WP], F16, tag="slab")
            sv0 = slab0[:].rearrange("p (g j) -> p g j", j=WP)
            QG = G // 4
            with tc.high_priority():
                # disjoint load ranges (no WAR chains between quarters)
                for quarter in range(4):
                    ga = 0 if quarter == 0 else quarter * QG + 1
                    gb = (quarter + 1) * QG + 1
                    leng = nc.sync if quarter < 2 else nc.gpsimd
                    for s in range(2):
                        src = bass.AP(
                            x_d.tensor,
                            (2 * ga + s) * xe_h,
                            [[xe_c, CI], [2 * xe_h, gb - ga], [1, WP]],
                        )
                        leng.dma_start(
                            slab0[:][16 * s : 16 * s + 16,
                                     ga * WP : gb * WP],
                            src,
                        )
                    if quarter == 0:
                        nc.sync.dma_start(wsb[0:128, 0:128], w_d[0:128, :])
                        nc.sync.dma_start(wsb[64:128, 128:256],
                                          w_d[128:192, :])
                for quarter in range(4):
                    copies(sv0, quarter * QG, (quarter + 1) * QG)
            slabs = {steps[0]: slab0}
            for k in (1, 2):
                slabs[steps[k]] = load_slab(*steps[k], eng=nc.gpsimd)
                if k == 1:
                    copies(slabs[steps[k]][:].rearrange(
                        "p (g j) -> p g j", j=WP), 0, G)
            for i, (n, t) in enumerate(steps):
                if i + 3 < len(steps):
                    slabs[steps[i + 3]] = load_slab(*steps[i + 3])
                nxt = (slabs[steps[i + 2]][:].rearrange(
                           "p (g j) -> p g j", j=WP)
                       if i + 2 < len(steps) else None)
                last = i == len(steps) - 1
                slab = slabs.pop((n, t))
                kw = dict(all_scalar=(i == 0), fine_stores=last,
                          dve_evacs=(1, 3, 5, 7) if last else (3, 6))
                if nxt is not None:
                    # DVE stream: c1(i+2), evac(i,3), c23(i+2), evac(i,6)
                    nc.vector.tensor_copy(
                        nxt[32:64, 0:G, 0:WP],
                        nxt[0:32, 1 : G + 1, 0:WP],
                    )
                    compute(n, t, slab, ebs=range(2), **kw)
                    nc.vector.tensor_copy(
                        nxt[64:128, 0:G, 0 : WP - 1],
                        nxt[0:64, 0:G, 1:WP],
                    )
                    compute(n, t, slab, ebs=range(2, 4), **kw)
                else:
                    compute(n, t, slab, **kw)

    nc.compile()
    return nc


def _prep_weights(W: np.ndarray) -> np.ndarray:
    # lhsT layouts, stored stacked as [192, 128] then transposed on load:
    #   w1[(d,s,ci), (ph,co)] = W[co, ci, s-ph, d]    (rows 0..127)
    #   w2[(s,ci),   (ph,co)] = W[co, ci, s-ph, 2]    (rows 128..191)
    w = np.zeros((192, 128), dtype=np.float32)
    for s in range(4):
        for ph in range(2):
            kh = s - ph
            if not (0 <= kh <= 2):
                continue
            blk = W[:, :, kh, :]  # [co, ci, kw]
            for d in range(2):
                w[d * 64 + s * 16 : d * 64 + (s + 1) * 16,
                  ph * 64 : (ph + 1) * 64] = blk[:, :, d].T
            w[128 + s * 16 : 128 + (s + 1) * 16,
              ph * 64 : (ph + 1) * 64] = blk[:, :, 2].T
    return w.astype(np.float16)


def _prep_inputs(x: np.ndarray, W: np.ndarray) -> list[dict]:
    wts = _prep_weights(np.asarray(W, dtype=np.float32))
    xs = np.asarray(x, dtype=np.float32).reshape(NCORES, NB, CI, H, W_SP)
    in_maps = []
    for i in range(NCORES):
        xp = np.zeros((NB, CI, HP, WP), dtype=np.float16)
        xp[:, :, 1 : H + 1, 1 : W_SP + 1] = xs[i]
        in_maps.append({"xp": xp, "wts": wts})
    return in_maps


def kernel(x: np.ndarray, W: np.ndarray) -> np.ndarray:
    assert x.shape == (N_FULL, CI, H, W_SP) and W.shape == (CO, CI, 3, 3)
    # BASS_TRACE without the axon NTFF hook module would crash the run path;
    # disable tracing only when the hook is genuinely unavailable.
    try:
        import antenv.axon_hooks  # noqa: F401
    except Exception:
        import os

        os.environ.setdefault("BASS_NEVER_TRACE", "1")
    if "nc" not in _CACHE:
        _CACHE["nc"] = _build()
    nc = _CACHE["nc"]

    in_maps = _prep_inputs(x, W)
    res = run_bass_kernel_spmd(nc, in_maps, list(range(NCORES)))
    parts = []
    for i in range(NCORES):
        dev = np.asarray(res.results[i]["out"], dtype=np.float32)
        # [n, t, eb, ph, co, h, gi, j] -> [n, co, (t eb h gi ph), j]
        # (out row = 64t + 16eb + 4h + 2gi + ph)
        dev = dev.reshape(NB, NSS, 4, 2, CO, 4, 2, 256)
        dev = dev.transpose(0, 4, 1, 2, 5, 6, 3, 7).reshape(NB, CO, H, W_SP)
        parts.append(dev)
    return np.concatenate(parts, axis=0)


# revision 72
# speedup vs baseline: 1.8771x; 1.0580x over previous
"""Trainium2 Bass kernel: 3x3 conv (N=16, C_in=16, C_out=64, H=W=256, pad=1).

Strategy (8 NeuronCores, data-parallel over batch N -> 2 images/core):
  - All device I/O in fp16 (host converts): DMA floor ~21MB/core.
  - Per 64-row strip: slab [128 partitions = (kw d in {0,1}, row-slot s in
    {0..3}, ci)] holds 4-row groups at stride 2; partition (d,s,ci) at free
    (g, j) = xpad[ci, h0+2g+s-1, j+d].  One HBM load fills (d=0, s in {0,1});
    three DVE copies (4x perf mode) build the rest from it.
  - 2 matmul passes per psum tile (vs 3 in the kh-block scheme): pass1
    contracts all 128 partitions (kw0+kw1 taps), pass2 contracts the kw1
    block read at +1px (= kw2 taps).  M = 128 = (row-phase ph in {0,1}) x
    64 channels -> 131072 psum rows total = PE structural floor.
  - PSUM -> SBUF evac with fp32->fp16 convert split across Scalar/GpSimd;
    batched 64-partition stores (even/odd output rows).
"""

import sys

if "/opt/trn_rl_repo" not in sys.path:
    sys.path.insert(0, "/opt/trn_rl_repo")

import numpy as np

import concourse.bacc as bacc
import concourse.bass as bass
import concourse.mybir as mybir
import concourse.tile as tile
from concourse.bass_utils import run_bass_kernel_spmd

N_FULL, CI, CO, H, W_SP = 16, 16, 64, 256, 256
NCORES = 8
NB = N_FULL // NCORES          # images per core
HP, WP = H + 2, W_SP + 2       # padded image dims (258)
RSTRIP = 64                    # output rows per strip
NSS = H // RSTRIP              # strips per image (4)
G = RSTRIP // 2                # row-pairs per strip (32)
GH = G + 1                     # loaded groups (incl. halo group)
NT = G // 2                    # psum tiles per strip (16)
F32 = mybir.dt.float32
F16 = mybir.dt.float16

_CACHE = {}


def _build():
    nc = bacc.Bacc("TRN2", target_bir_lowering=False, debug=False)
    EB = 8                     # psum tiles per evac batch (32 output rows)
    NEB = NT // EB             # evac batches per strip (2)

    x_d = nc.dram_tensor("xp", [NB, CI, HP, WP], F16, kind="ExternalInput").ap()
    w_d = nc.dram_tensor("wts", [192, 128], F16, kind="ExternalInput").ap()
    # device-layout output: [n, strip, batch, (ph,co), (t', gi, j)];
    # host un-permutes to NCHW (out row = 64t + 32b + 4t' + 2gi + ph)
    o_d = nc.dram_tensor(
        "out", [NB, NSS, NEB, 128, EB * 512], F16, kind="ExternalOutput"
    ).ap()

    xe_n = CI * HP * WP        # x_pad element strides
    xe_c = HP * WP
    xe_h = WP

    with tile.TileContext(nc) as tc:
        with (
            tc.tile_pool(name="wp", bufs=1) as wpool,
            tc.tile_pool(name="slab", bufs=4) as slabpool,
            tc.tile_pool(name="evac", bufs=4) as evacpool,
            tc.tile_pool(name="ps", bufs=4, space="PSUM") as pspool,
        ):
            wsb = wpool.tile([128, 256], F16)
            # w1 = wsb[:, 0:128] (pass1, all 128 partitions)
            # w2 = wsb[64:128, 128:256] (pass2, kw1 block partitions)
            # (loads emitted in the prologue, after slab0's loads)

            def copies(sv, g0, g1, pool_c1=0):
                # c1:  kw0_s23[g] = kw0_s01[g+1]      (rows 2g+1, 2g+2)
                #      (optionally first `pool_c1` groups on GpSimd)
                # c23: kw1[g,j]   = kw0[g,j+1]        (64-partition copy)
                if pool_c1:
                    nc.gpsimd.tensor_copy(
                        sv[32:64, g0 : g0 + pool_c1, 0:WP],
                        sv[0:32, g0 + 1 : g0 + pool_c1 + 1, 0:WP],
                    )
                nc.vector.tensor_copy(
                    sv[32:64, g0 + pool_c1 : g1, 0:WP],
                    sv[0:32, g0 + pool_c1 + 1 : g1 + 1, 0:WP],
                )
                nc.vector.tensor_copy(
                    sv[64:128, g0:g1, 0 : WP - 1],
                    sv[0:64, g0:g1, 1:WP],
                )

            def load_slab(n, t, eng=None):
                # slab covers the whole strip: groups 0..G-1 (+ halo slot G)
                slab = slabpool.tile([128, GH * WP], F16, tag="slab")
                sf = slab[:]
                for s in range(2):
                    src = bass.AP(
                        x_d.tensor,
                        n * xe_n + (RSTRIP * t + s) * xe_h,
                        [[xe_c, CI], [2 * xe_h, GH], [1, WP]],
                    )
                    (eng or nc.sync).dma_start(sf[16 * s : 16 * s + 16, :], src)
                return slab

            def load_slab_extras(slab, n, t, eng):
                # fill s23 / kw1 blocks straight from HBM (no DVE copies);
                # only worthwhile while the DMA queue still has slack
                sf = slab[:]
                base = n * xe_n + RSTRIP * t * xe_h
                for s in (2, 3):     # kw0_s23: x-rows 2g+s-1, g in 0..G-1
                    src = bass.AP(
                        x_d.tensor,
                        base + s * xe_h,
                        [[xe_c, CI], [2 * xe_h, G], [1, WP]],
                    )
                    eng.dma_start(
                        sf[16 * s : 16 * s + 16, 0 : G * WP], src
                    )
                sv = sf.rearrange("p (g j) -> p g j", j=WP)
                for s in range(4):   # kw1: same rows, +1 px, 257 cols
                    src = bass.AP(
                        x_d.tensor,
                        base + s * xe_h + 1,
                        [[xe_c, CI], [2 * xe_h, G], [1, WP - 1]],
                    )
                    eng.dma_start(
                        sv[64 + 16 * s : 80 + 16 * s, 0:G, 0 : WP - 1], src
                    )

            def compute(n, t, slab, all_scalar=False, fine_stores=False,
                        dve_evacs=(3, 7), ebs=range(4)):
                sv = slab[:].rearrange("p (g j) -> p g j", j=WP)
                for eb in ebs:
                    evac = evacpool.tile([128, 2048], F16, tag="evac")
                    for pth in range(2):
                        pt = 2 * eb + pth
                        ps = pspool.tile([128, 1024], F32, tag="ps")
                        for q in range(2):
                            g0 = 2 * (2 * pt + q)
                            nc.tensor.matmul(
                                ps[:, q * 512 : (q + 1) * 512],
                                wsb[0:128, 0:128],
                                sv[0:128, g0 : g0 + 2, 0:256],
                                start=True,
                                stop=False,
                            )
                            nc.tensor.matmul(
                                ps[:, q * 512 : (q + 1) * 512],
                                wsb[64:128, 128:256],
                                sv[64:128, g0 : g0 + 2, 1:257],
                                start=False,
                                stop=True,
                            )
                        dve = (pt in dve_evacs) and not all_scalar
                        op = nc.vector.tensor_copy if dve else nc.scalar.copy
                        op(evac[:, pth * 1024 : (pth + 1) * 1024], ps[:])
                        if fine_stores:
                            dst = bass.AP(
                                o_d.tensor,
                                (((n * NSS + t) * 4 + eb) * 128 * 2048
                                 + pth * 1024),
                                [[2048, 128], [1, 1024]],
                            )
                            nc.sync.dma_start(
                                dst, evac[:, pth * 1024 : (pth + 1) * 1024]
                            )
                    if not fine_stores:
                        dst = bass.AP(
                            o_d.tensor,
                            ((n * NSS + t) * 4 + eb) * 128 * 2048,
                            [[2048, 128], [1, 2048]],
                        )
                        nc.sync.dma_start(dst, evac[:])

            # --- software pipeline ---------------------------------------
            # Loads run 2 steps ahead (SP queue, ahead of stores); copies run
            # 1 step ahead and are emitted BEFORE each step's evacs so the
            # DVE never idles on matmul waits while copies are ready.
            steps = [(n, t) for n in range(NB) for t in range(NSS)]

            # prologue: slab 0 in quarters for a fast start (dep tracking is
            # region-precise, so early matmuls run on partial slabs);
            # high_priority pins these ahead of later copies in the scheduler
            slab0 = slabpool.tile([128, GH * WP], F16, tag="slab")
            sv0 = slab0[:].rearrange("p (g j) -> p g j", j=WP)
            QG = G // 4
            with tc.high_priority():
                # disjoint load ranges (no WAR chains between quarters)
                for quarter in range(4):
                    ga = 0 if quarter == 0 else quarter * QG + 1
                    gb = (quarter + 1) * QG + 1
                    leng = nc.sync if quarter < 2 else nc.gpsimd
                    for s in range(2):
                        src = bass.AP(
                            x_d.tensor,
                            (2 * ga + s) * xe_h,
                            [[xe_c, CI], [2 * xe_h, gb - ga], [1, WP]],
                        )
                        leng.dma_start(
                            slab0[:][16 * s : 16 * s + 16,
                                     ga * WP : gb * WP],
                            src,
                        )
                    if quarter == 0:
                        nc.sync.dma_start(wsb[0:128, 0:128], w_d[0:128, :])
                        nc.sync.dma_start(wsb[64:128, 128:256],
                                          w_d[128:192, :])
                for quarter in range(4):
                    copies(sv0, quarter * QG, (quarter + 1) * QG)
            slabs = {steps[0]: slab0}
            slabs[steps[2]] = load_slab(*steps[2], eng=nc.sync)
            slabs[steps[1]] = load_slab(*steps[1], eng=nc.gpsimd)
            load_slab_extras(slabs[steps[1]], *steps[1], nc.sync)
            for i, (n, t) in enumerate(steps):
                if i + 3 < len(steps):
                    slabs[steps[i + 3]] = load_slab(*steps[i + 3])
                nxt = (slabs[steps[i + 2]][:].rearrange(
                           "p (g j) -> p g j", j=WP)
                       if i + 2 < len(steps) else None)
                last = i == len(steps) - 1
                slab = slabs.pop((n, t))
                kw = dict(all_scalar=(i == 0), fine_stores=last,
                          dve_evacs=(1, 3, 5, 7) if last else (3, 6))
                if nxt is not None:
                    # DVE stream: c1(i+2), evac(i,3), c23(i+2), evac(i,6)
                    nc.vector.tensor_copy(
                        nxt[32:64, 0:G, 0:WP],
                        nxt[0:32, 1 : G + 1, 0:WP],
                    )
                    compute(n, t, slab, ebs=range(2), **kw)
                    nc.vector.tensor_copy(
                        nxt[64:128, 0:G, 0 : WP - 1],
                        nxt[0:64, 0:G, 1:WP],
                    )
                    compute(n, t, slab, ebs=range(2, 4), **kw)
                else:
                    compute(n, t, slab, **kw)

    nc.compile()
    return nc


def _prep_weights(W: np.ndarray) -> np.ndarray:
    # lhsT layouts, stored stacked as [192, 128] then transposed on load:
    #   w1[(d,s,ci), (ph,co)] = W[co, ci, s-ph, d]    (rows 0..127)
    #   w2[(s,ci),   (ph,co)] = W[co, ci, s-ph, 2]    (rows 128..191)
    w = np.zeros((192, 128), dtype=np.float32)
    for s in range(4):
        for ph in range(2):
            kh = s - ph
            if not (0 <= kh <= 2):
                continue
            blk = W[:, :, kh, :]  # [co, ci, kw]
            for d in range(2):
                w[d * 64 + s * 16 : d * 64 + (s + 1) * 16,
                  ph * 64 : (ph + 1) * 64] = blk[:, :, d].T
            w[128 + s * 16 : 128 + (s + 1) * 16,
              ph * 64 : (ph + 1) * 64] = blk[:, :, 2].T
    return w.astype(np.float16)


def _prep_inputs(x: np.ndarray, W: np.ndarray) -> list[dict]:
    wts = _prep_weights(np.asarray(W, dtype=np.float32))
    xs = np.asarray(x, dtype=np.float32).reshape(NCORES, NB, CI, H, W_SP)
    in_maps = []
    for i in range(NCORES):
        xp = np.zeros((NB, CI, HP, WP), dtype=np.float16)
        xp[:, :, 1 : H + 1, 1 : W_SP + 1] = xs[i]
        in_maps.append({"xp": xp, "wts": wts})
    return in_maps


def kernel(x: np.ndarray, W: np.ndarray) -> np.ndarray:
    assert x.shape == (N_FULL, CI, H, W_SP) and W.shape == (CO, CI, 3, 3)
    # BASS_TRACE without the axon NTFF hook module would crash the run path;
    # disable tracing only when the hook is genuinely unavailable.
    try:
        import antenv.axon_hooks  # noqa: F401
    except Exception:
        import os

        os.environ.setdefault("BASS_NEVER_TRACE", "1")
    if "nc" not in _CACHE:
        _CACHE["nc"] = _build()
    nc = _CACHE["nc"]

    in_maps = _prep_inputs(x, W)
    res = run_bass_kernel_spmd(nc, in_maps, list(range(NCORES)))
    parts = []
    for i in range(NCORES):
        dev = np.asarray(res.results[i]["out"], dtype=np.float32)
        # [n, t, eb, ph, co, h, gi, j] -> [n, co, (t eb h gi ph), j]
        # (out row = 64t + 16eb + 4h + 2gi + ph)
        dev = dev.reshape(NB, NSS, 4, 2, CO, 4, 2, 256)
        dev = dev.transpose(0, 4, 1, 2, 5, 6, 3, 7).reshape(NB, CO, H, W_SP)
        parts.append(dev)
    return np.concatenate(parts, axis=0)
